# revision 1
# baseline (speedup 1.0000x reference)
"""Trainium2 Bass kernel for a 6-layer post-LN transformer encoder.

Model: V=32000, D=1024, L=6, H=16, HD=64, F=4096, B=4, S=2048 (fp32).

Sharding: 8 NeuronCores, token-parallel. Core c owns batch b=c//2,
sequence half c%2 (1024 tokens). Per layer, K^T (feature-major) and V
(token-major) for the local tokens are computed locally, then exchanged
via an 8-rank AllGather; each core slices out its batch's blocks with
partition-id-derived dynamic DMA offsets.

Layouts: activations are feature-major ("x^T": [D partitions, tokens]) so
every projection contracts over partitions. Scores are computed
transposed (S^T = K^T_slice.T @ Q^T_slice per head, [keys, q]), softmax
sums come from a ones-column appended to V (attnT_aug = [V|1].T @ P^T),
and per-head normalization uses gpsimd partition_broadcast of the
reciprocal sums. All matmul operands are float32r (full-rate fp32 on the
PE at moving-dim >= 256, ~1e-4 component error).
"""
import sys

sys.path.insert(0, "/opt/trn_rl_repo")

from contextlib import ExitStack

import numpy as np

import concourse.bass as bass
import concourse.tile as tile
from concourse import bacc, mybir
from concourse.bass import ds
from concourse.bass_utils import run_bass_kernel_spmd
from concourse.masks import make_identity

P = 128
V, D, L, H, F, M = 32000, 1024, 6, 16, 4096, 2048
B, S = 4, 2048
HD = D // H
EPS = 1e-5
NCC = 8
T = 1024            # tokens per core
DT = D // P         # 8 feature tiles
FT = F // P         # 32 ff tiles
KT = 2048 // P      # 16 key tiles (full batch sequence)
NQ = 2              # 512-token chunks per core
f32 = mybir.dt.float32
f32r = mybir.dt.float32r
i32 = mybir.dt.int32
AF = mybir.ActivationFunctionType
OP = mybir.AluOpType

_cache = {}


def _build(nl, stop_after=99, skip=(), tune=()):
    nc = bacc.Bacc(None, num_devices=NCC)

    # ---- DRAM I/O ----
    tok_emb = nc.dram_tensor("tok_emb", [V, D], f32, kind="ExternalInput")
    src_idx = nc.dram_tensor("src_idx", [P, DT], i32, kind="ExternalInput")
    pos = nc.dram_tensor("pos", [T, D], f32, kind="ExternalInput")
    mbt = nc.dram_tensor("mbt", [P, KT], f32, kind="ExternalInput")
    Wq = nc.dram_tensor("Wq", [nl, D, D], f32r, kind="ExternalInput")
    Wk = nc.dram_tensor("Wk", [nl, D, D], f32r, kind="ExternalInput")
    Wv = nc.dram_tensor("Wv", [nl, D, D], f32r, kind="ExternalInput")
    Wo = nc.dram_tensor("Wo", [nl, D, D], f32r, kind="ExternalInput")
    W1 = nc.dram_tensor("W1", [nl, D, F], f32r, kind="ExternalInput")
    W2 = nc.dram_tensor("W2", [nl, F, D], f32r, kind="ExternalInput")
    # per-feature scalars, laid out [128, nl*ntiles] (col = l*ntiles + tile)
    bq_a = nc.dram_tensor("bq_a", [P, nl * DT], f32, kind="ExternalInput")
    bk_a = nc.dram_tensor("bk_a", [P, nl * DT], f32, kind="ExternalInput")
    bo_a = nc.dram_tensor("bo_a", [P, nl * DT], f32, kind="ExternalInput")
    b2_a = nc.dram_tensor("b2_a", [P, nl * DT], f32, kind="ExternalInput")
    g1_a = nc.dram_tensor("g1_a", [P, nl * DT], f32, kind="ExternalInput")
    c1_a = nc.dram_tensor("c1_a", [P, nl * DT], f32, kind="ExternalInput")
    g2_a = nc.dram_tensor("g2_a", [P, nl * DT], f32, kind="ExternalInput")
    c2_a = nc.dram_tensor("c2_a", [P, nl * DT], f32, kind="ExternalInput")
    b1_a = nc.dram_tensor("b1_a", [P, nl * FT], f32, kind="ExternalInput")

    xT_out = nc.dram_tensor("xT_out", [D, T], f32, kind="ExternalOutput")

    with tile.TileContext(nc) as tc, ExitStack() as ctx:
        sb = ctx.enter_context(tc.tile_pool(name="sb", bufs=1))
        pp = ctx.enter_context(tc.tile_pool(name="pp", bufs=1, space="PSUM"))
        dr = ctx.enter_context(tc.tile_pool(name="dr", bufs=1, space="DRAM"))
        sing = ctx.enter_context(tc.tile_pool(name="sing", bufs=1))

        def big(dt=f32r):
            return sb.tile([P, T], dt, name="big", tag="big", bufs=18)

        def c512(shape=(P, 512), dt=f32r, tag="c512", bufs=4):
            return sb.tile(list(shape), dt, name=tag, tag=tag, bufs=bufs)

        def psum(tag="proj", bufs=3, shape=(P, 512)):
            if "pb" in tune and tag == "proj":
                bufs = 2
            if "pb" in tune and tag == "av":
                bufs = 3
            return pp.tile(list(shape), f32, space="PSUM", name=tag, tag=tag, bufs=bufs)

        pid = nc.sync.partition_id()
        r0row = (pid // 2) * (2 * 2 * T)     # row base of this batch's rank-0 block

        # ---- constants ----
        ident = sing.tile([P, P], f32)
        make_identity(nc, ident)
        ones_s = sing.tile([P, 1], f32r)
        nc.vector.memset(ones_s.bitcast(f32), 1.0 / D)
        eps_s = sing.tile([1, 1], f32)
        nc.vector.memset(eps_s, EPS)
        mb_s = sing.tile([P, KT], f32)
        nc.sync.dma_start(out=mb_s, in_=mbt[:, :])
        bias = {}
        for name, t_ in (("bq", bq_a), ("bk", bk_a), ("bo", bo_a), ("b2", b2_a),
                         ("g1", g1_a), ("c1", c1_a), ("g2", g2_a), ("c2", c2_a)):
            s = sing.tile([P, nl * DT], f32, name=f"bias_{name}")
            nc.sync.dma_start(out=s, in_=t_[:, :])
            bias[name] = s
        b1_s = sing.tile([P, nl * FT], f32)
        nc.sync.dma_start(out=b1_s, in_=b1_a[:, :])

        # ---- embedding: gather + scale + pos, then transpose to xT ----
        idx_s = sing.tile([P, DT], i32)
        nc.sync.dma_start(out=idx_s, in_=src_idx[:, :])
        x0_tiles = []
        for c in range(DT):
            emb_s = big(f32)
            nc.gpsimd.indirect_dma_start(
                out=emb_s[:, :], out_offset=None,
                in_=tok_emb[:, :],
                in_offset=bass.IndirectOffsetOnAxis(ap=idx_s[:, c:c + 1], axis=0),
            )
            pos_s = big(f32)
            nc.sync.dma_start(out=pos_s, in_=pos[c * P:(c + 1) * P, :])
            x0 = big(f32)
            nc.vector.scalar_tensor_tensor(
                out=x0, in0=emb_s, scalar=float(np.sqrt(D)), in1=pos_s,
                op0=OP.mult, op1=OP.add)
            x0_tiles.append(x0)
        xT = []
        for m in range(DT):
            t_ = big()
            for c in range(DT):
                ps = psum()
                nc.tensor.transpose(ps[:, 0:P], x0_tiles[c][:, m * P:(m + 1) * P], ident)
                nc.vector.tensor_copy(t_[:, c * P:(c + 1) * P], ps[:, 0:P])
            xT.append(t_)

        for l in range(nl if stop_after >= 1 else 0):
            bcol = l * DT

            def wload(wt, mi, n_kt=DT, name="wblk"):
                # weight block [128, n_kt, 128] = wt[l][:, mi*128:(mi+1)*128]
                blk = sb.tile([P, n_kt, P], f32r, name=name, tag="wblk", bufs=2)
                src = wt[l].rearrange("(kt p) c -> p kt c", p=P)[:, :, mi * P:(mi + 1) * P]
                nc.sync.dma_start(out=blk, in_=src)
                return blk

            kv = dr.tile([2 * T, D], f32r, name="kv", tag="kv", bufs=2)
            ag = dr.tile([NCC * 2 * T, D], f32r, name="ag", tag="ag", bufs=2,
                         addr_space="Shared")

            # ---- K^T projection -> kv[0:T] (feature-major) ----
            for m in range(DT if "kv" not in skip else 0):
                kb = wload(Wk, m)
                ps2 = [psum() for _ in range(NQ)]
                for k in range(DT):
                    for n in range(NQ):
                        nc.tensor.matmul(ps2[n], kb[:, k, :], xT[k][:, n * 512:(n + 1) * 512],
                                         start=(k == 0), stop=(k == DT - 1))
                for n in range(NQ):
                    kchunk = c512()
                    nc.vector.tensor_scalar_add(kchunk, ps2[n], bias["bk"][:, bcol + m:bcol + m + 1])
                    nc.sync.dma_start(out=kv[m * P:(m + 1) * P, n * 512:(n + 1) * 512],
                                      in_=kchunk)
            # ---- V projection -> kv[T:2T] (token-major) ----
            vbs = {}
            for nf in range(NQ if "kv" not in skip else 0):
                for k in range(DT):
                    vb = c512(tag="ht", bufs=34)
                    nc.sync.dma_start(out=vb, in_=Wv[l][k * P:(k + 1) * P,
                                                        nf * 512:(nf + 1) * 512])
                    vbs[nf, k] = vb
            for tt in range(DT if "kv" not in skip else 0):
                ps2 = [psum() for _ in range(NQ)]
                for k in range(DT):
                    for nf in range(NQ):
                        nc.tensor.matmul(ps2[nf], xT[k][:, tt * P:(tt + 1) * P], vbs[nf, k],
                                         start=(k == 0), stop=(k == DT - 1))
                for nf in range(NQ):
                    vchunk = c512()
                    nc.vector.tensor_copy(vchunk, ps2[nf])
                    nc.sync.dma_start(out=kv[T + tt * P:T + (tt + 1) * P,
                                             nf * 512:(nf + 1) * 512],
                                      in_=vchunk)
            if stop_after <= 2:
                break
            # ---- AllGather K^T/V across all 8 cores ----
            if "kv" in skip:
                pass
            else:
                nc.gpsimd.collective_compute(
                "AllGather", OP.bypass,
                replica_groups=[list(range(NCC))],
                    ins=[kv[:, :]],
                    outs=[ag[:, :]],
                )

            if stop_after <= 3:
                break
            # ---- attention ----
            attnT = [big() for _ in range(DT)]
            if "attn" in skip:
                for t_ in attnT:
                    nc.vector.memset(t_.bitcast(f32), 0.01)
            for hp in range(DT if "attn" not in skip else 0):  # head pairs
                ktp = sb.tile([P, 2 * T], f32r, name="ktp", tag="ktp", bufs=2)
                nc.sync.dma_start(out=ktp[:, 0:T], in_=ag[ds(r0row + hp * P, P), :])
                nc.sync.dma_start(out=ktp[:, T:2 * T],
                                  in_=ag[ds(r0row + 2 * T + hp * P, P), :])
                vags = []
                for hh in range(2):
                    h = 2 * hp + hh
                    va = sb.tile([P, KT, HD + 1], f32r, name="vag", tag="vag", bufs=3)
                    for ri in range(2):
                        src = ag[ds(r0row + ri * 2 * T + T, T),
                                 h * HD:(h + 1) * HD].rearrange("(kt p) c -> p kt c", p=P)
                        nc.sync.dma_start(out=va[:, ri * DT:(ri + 1) * DT, 0:HD], in_=src)
                    nc.vector.memset(va.bitcast(f32)[:, :, HD:HD + 1], 1.0)
                    vags.append(va)
                qb = wload(Wq, hp)
                qps = [psum() for _ in range(NQ)]
                for k in range(DT):
                    for qt in range(NQ):
                        nc.tensor.matmul(qps[qt], qb[:, k, :],
                                         xT[k][:, qt * 512:(qt + 1) * 512],
                                         start=(k == 0), stop=(k == DT - 1))
                qcs = []
                for qt in range(NQ):
                    qc = c512()
                    nc.vector.tensor_scalar_add(qc, qps[qt], bias["bq"][:, bcol + hp:bcol + hp + 1])
                    qcs.append(qc)
                for qt in range(NQ):
                    qsl = slice(qt * 512, (qt + 1) * 512)
                    qc = qcs[qt]

                    av_ps = [psum(tag="av", bufs=2, shape=(HD + 1, 512)) for _ in range(2)]
                    prev = None
                    for kt in range(KT):
                        sts = [psum(tag="st", bufs=3) for _ in range(2)]
                        ksl = slice(kt * P, (kt + 1) * P)
                        nc.tensor.matmul(sts[0], ktp[0:HD, ksl], qc[0:HD, :],
                                         start=True, stop=True, tile_position=(0, 0))
                        nc.tensor.matmul(sts[1], ktp[HD:P, ksl], qc[HD:P, :],
                                         start=True, stop=True, tile_position=(64, 0))
                        cur = []
                        for hh in range(2):
                            pr = c512(tag="probs", bufs=8 if "pb" in tune else 4)
                            if "nobias" in tune:
                                nc.scalar.activation(out=pr, in_=sts[hh], func=AF.Exp,
                                                     scale=float(1.0 / np.sqrt(HD)))
                            else:
                                nc.scalar.activation(out=pr, in_=sts[hh], func=AF.Exp,
                                                     bias=mb_s[:, kt:kt + 1],
                                                     scale=float(1.0 / np.sqrt(HD)))
                            cur.append(pr)
                        # AV lags ST by one kt to hide exp latency
                        if prev is not None:
                            pkt = kt - 1
                            for hh in range(2):
                                nc.tensor.matmul(av_ps[hh], vags[hh][:, pkt, :], prev[hh],
                                                 start=(pkt == 0), stop=False)
                        prev = cur
                    for hh in range(2):
                        nc.tensor.matmul(av_ps[hh], vags[hh][:, KT - 1, :], prev[hh],
                                         start=False, stop=True)
                    for hh in range(2):
                        if "nonorm" in tune:
                            nc.vector.tensor_copy(attnT[hp][hh * HD:(hh + 1) * HD, qsl],
                                                  av_ps[hh][0:HD, :])
                            continue
                        rs = sb.tile([1, 512], f32, name="rs", tag="rows", bufs=4)
                        nc.vector.reciprocal(rs, av_ps[hh][HD:HD + 1, :])
                        rb = sb.tile([HD, 512], f32, name="rb", tag="rb", bufs=2)
                        nc.gpsimd.partition_broadcast(rb, rs[0:1, :], channels=HD)
                        nc.vector.tensor_tensor(
                            out=attnT[hp][hh * HD:(hh + 1) * HD, qsl],
                            in0=av_ps[hh][0:HD, :], in1=rb, op=OP.mult)

            if stop_after <= 4:
                break
            # ---- O projection + residual (in-place into xT) ; LN1 ----
            for m in range(DT):
                ob = wload(Wo, m)
                ps2 = [psum() for _ in range(NQ)]
                for k in range(DT):
                    for n in range(NQ):
                        nc.tensor.matmul(ps2[n], ob[:, k, :], attnT[k][:, n * 512:(n + 1) * 512],
                                         start=(k == 0), stop=(k == DT - 1))
                for n in range(NQ):
                    nc.vector.scalar_tensor_tensor(
                        out=xT[m][:, n * 512:(n + 1) * 512], in0=ps2[n],
                        scalar=bias["bo"][:, bcol + m:bcol + m + 1],
                        in1=xT[m].bitcast(f32)[:, n * 512:(n + 1) * 512],
                        op0=OP.add, op1=OP.add)
            _layernorm(nc, big, c512, psum, sb, xT, ones_s, eps_s,
                       bias["g1"], bias["c1"], bcol)

            if stop_after <= 5:
                break
            # ---- FFN (F in halves; W1/W2 weights reused across both token chunks) ----
            u2 = []
            if "ffn" in skip:
                for m in range(DT):
                    u2.append(big())
                    nc.vector.tensor_copy(u2[m], xT[m].bitcast(f32))
            for fhalf in range(2 if "ffn" not in skip else 0):
                hT = {}
                for fi in range(FT // 2):
                    fm = fhalf * (FT // 2) + fi
                    w1b = wload(W1, fm)
                    ps2 = [psum() for _ in range(NQ)]
                    for k in range(DT):
                        for tc2 in range(NQ):
                            nc.tensor.matmul(ps2[tc2], w1b[:, k, :],
                                             xT[k][:, tc2 * 512:(tc2 + 1) * 512],
                                             start=(k == 0), stop=(k == DT - 1))
                    for tc2 in range(NQ):
                        ht = c512(tag="ht", bufs=34)
                        nc.vector.tensor_scalar(
                            out=ht, in0=ps2[tc2],
                            scalar1=b1_s[:, l * FT + fm:l * FT + fm + 1], scalar2=0.0,
                            op0=OP.add, op1=OP.max)
                        hT[tc2, fi] = ht
                for m in range(DT):
                    if fhalf == 0:
                        u2.append(big())
                    ps2 = [psum() for _ in range(NQ)]
                    for quarter in range(2):
                        w2b = sb.tile([P, FT // 4, P], f32r, name="wblk", tag="wblk", bufs=2)
                        nc.sync.dma_start(
                            out=w2b,
                            in_=W2[l].rearrange("(kt p) c -> p kt c", p=P)[
                                :, fhalf * (FT // 2) + quarter * (FT // 4):
                                   fhalf * (FT // 2) + (quarter + 1) * (FT // 4),
                                m * P:(m + 1) * P])
                        for kk in range(FT // 4):
                            kl = quarter * (FT // 4) + kk
                            for tc2 in range(NQ):
                                nc.tensor.matmul(ps2[tc2], w2b[:, kk, :], hT[tc2, kl],
                                                 start=(kl == 0), stop=(kl == FT // 2 - 1))
                    for tc2 in range(NQ):
                        tsl = slice(tc2 * 512, (tc2 + 1) * 512)
                        if fhalf == 0:
                            nc.vector.scalar_tensor_tensor(
                                out=u2[m][:, tsl], in0=ps2[tc2],
                                scalar=bias["b2"][:, bcol + m:bcol + m + 1],
                                in1=xT[m].bitcast(f32)[:, tsl],
                                op0=OP.add, op1=OP.add)
                        else:
                            nc.vector.tensor_add(u2[m][:, tsl], ps2[tc2],
                                                 u2[m].bitcast(f32)[:, tsl])
            _layernorm(nc, big, c512, psum, sb, u2, ones_s, eps_s,
                       bias["g2"], bias["c2"], bcol)
            xT = u2

        for m in range(DT):
            nc.sync.dma_start(out=xT_out[m * P:(m + 1) * P, :], in_=xT[m].bitcast(f32))

    nc.finalize()
    return nc


def _layernorm(nc, big, c512, psum, sb, u, ones_s, eps_s, g_s, c_s, bcol):
    """Feature-axis layernorm over feature-major tiles u (list of DT [128, T])."""
    mean_b = big(f32)
    rstd_b = big(f32)
    for n in range(NQ):
        tsl = slice(n * 512, (n + 1) * 512)
        mps = psum()
        for k in range(DT):
            nc.tensor.matmul(mps[0:1, :], ones_s, u[k][:, tsl],
                             start=(k == 0), stop=(k == DT - 1))
        mean_r = sb.tile([1, 512], f32, name="mean_r", tag="rows", bufs=4)
        nc.vector.tensor_copy(mean_r, mps[0:1, :])
        e2ps = psum()
        for k in range(DT):
            usq = c512(tag="probs", bufs=4)
            nc.vector.tensor_mul(usq, u[k].bitcast(f32)[:, tsl], u[k].bitcast(f32)[:, tsl])
            nc.tensor.matmul(e2ps[0:1, :], ones_s, usq,
                             start=(k == 0), stop=(k == DT - 1))
        var_r = sb.tile([1, 512], f32, name="var_r", tag="rows", bufs=4)
        # var = E[u^2] - mean^2
        nc.vector.tensor_mul(var_r, mean_r, mean_r)
        nc.vector.tensor_sub(var_r, e2ps[0:1, :], var_r)
        sd_r = sb.tile([1, 512], f32, name="sd_r", tag="rows", bufs=4)
        nc.scalar.activation(out=sd_r, in_=var_r, func=AF.Sqrt, bias=eps_s[0:1, :])
        rstd_r = sb.tile([1, 512], f32, name="rstd_r", tag="rows", bufs=4)
        nc.vector.reciprocal(rstd_r, sd_r)
        nc.gpsimd.partition_broadcast(mean_b[:, tsl], mean_r[0:1, :], channels=P)
        nc.gpsimd.partition_broadcast(rstd_b[:, tsl], rstd_r[0:1, :], channels=P)
    for m in range(DT):
        nc.vector.tensor_sub(u[m], u[m].bitcast(f32), mean_b)
        nc.vector.tensor_mul(u[m], u[m].bitcast(f32), rstd_b)
        nc.vector.tensor_scalar(
            out=u[m], in0=u[m].bitcast(f32),
            scalar1=g_s[:, bcol + m:bcol + m + 1],
            scalar2=c_s[:, bcol + m:bcol + m + 1],
            op0=OP.mult, op1=OP.add)
    return u


def _prep_feat(x, nl, nt):
    # [nl, dim] -> [128, nl*nt] with col l*nt + tile
    return np.ascontiguousarray(
        np.asarray(x, np.float32).reshape(nl, nt, P).transpose(2, 0, 1).reshape(P, nl * nt))


def prepare_in_maps(src, src_mask, tok_emb, pos_emb, Wq, bq, Wk, bk, Wv, bv,
                    Wo, bo, ln1_g, ln1_b, W1, b1, W2, b2, ln2_g, ln2_b):
    nl = np.asarray(Wq).shape[0]
    src = np.asarray(src)
    src_mask = np.asarray(src_mask)
    assert not np.any(np.asarray(bv)), "nonzero bv not implemented"

    np32 = lambda x: np.ascontiguousarray(np.asarray(x, np.float32))
    common = dict(
        tok_emb=np32(tok_emb),
        Wq=np32(Wq), Wk=np32(Wk), Wv=np32(Wv), Wo=np32(Wo),
        W1=np32(W1), W2=np32(W2),
        bq_a=_prep_feat(bq, nl, DT), bk_a=_prep_feat(bk, nl, DT),
        bo_a=_prep_feat(bo, nl, DT), b2_a=_prep_feat(b2, nl, DT),
        g1_a=_prep_feat(ln1_g, nl, DT), c1_a=_prep_feat(ln1_b, nl, DT),
        g2_a=_prep_feat(ln2_g, nl, DT), c2_a=_prep_feat(ln2_b, nl, DT),
        b1_a=_prep_feat(b1, nl, FT),
    )
    in_maps = []
    for c in range(NCC):
        b, hf = c // 2, c % 2
        tok = src[b, hf * T:(hf + 1) * T].astype(np.int32)
        mask = np.asarray(src_mask[b, 0, 0], np.float32)
        mbias = np.where(mask == 0, np.float32(-1e10), np.float32(0.0))
        in_maps.append(dict(
            common,
            src_idx=np.ascontiguousarray(tok.reshape(DT, P).T),
            pos=np32(pos_emb[hf * T:(hf + 1) * T]),
            mbt=np.ascontiguousarray(mbias.reshape(KT, P).T),
        ))
    return nl, in_maps


def assemble_out(per_core_results):
    out = np.empty((B, S, D), np.float32)
    for c in range(NCC):
        b, hf = c // 2, c % 2
        out[b, hf * T:(hf + 1) * T] = per_core_results[c]["xT_out"].T
    return out


def kernel(**inputs):
    nl, in_maps = prepare_in_maps(**inputs)
    if "nc" not in _cache or _cache.get("nl") != nl:
        _cache["nc"] = _build(nl)
        _cache["nl"] = nl
    res = run_bass_kernel_spmd(_cache["nc"], in_maps, core_ids=list(range(NCC)))
    return assemble_out(res.results)



# revision 2
# speedup vs baseline: 39.1602x; 39.1602x over previous
"""Trainium2 Bass kernel for a 6-layer post-LN transformer encoder.

Model: V=32000, D=1024, L=6, H=16, HD=64, F=4096, B=4, S=2048 (fp32).

Sharding: 8 NeuronCores, token-parallel. Core c owns batch b=c//2,
sequence half c%2 (1024 tokens). Per layer, K^T (feature-major) and V
(token-major) for the local tokens are computed locally in bf16, then
exchanged with the same-batch partner core via 2-rank AllGathers
(replica groups [0,1],[2,3],[4,5],[6,7]) — K first so its gather
overlaps the V/Q projections, V's overlaps the score matmuls.

Layouts: activations are feature-major ("x^T": [D partitions, tokens]) so
every projection contracts over partitions. Weights are stored/loaded as
bf16 (halves HBM traffic; PE runs bf16 stationary x f32r moving at full
rate). Scores are computed transposed (S^T = K^T_slice.T @ Q^T_slice per
head, [keys, q]) with both heads of a pair running concurrently via PE
row tiling, softmax sums come from a ones-column appended to V
(attnT_aug = [V|1].T @ P^T), and per-head normalization uses gpsimd
partition_broadcast of the reciprocal sums.
"""
import sys

sys.path.insert(0, "/opt/trn_rl_repo")

from contextlib import ExitStack

import numpy as np

import concourse.bass as bass
import concourse.tile as tile
from concourse import bacc, mybir
from concourse.bass import ds
from concourse.bass_utils import run_bass_kernel_spmd
from concourse.masks import make_identity

P = 128
V, D, L, H, F, M = 32000, 1024, 6, 16, 4096, 2048
B, S = 4, 2048
HD = D // H
EPS = 1e-5
NCC = 8
T = 1024            # tokens per core
DT = D // P         # 8 feature tiles
FT = F // P         # 32 ff tiles
KT = 2048 // P      # 16 key tiles (full batch sequence)
NQ = 2              # 512-token chunks per core
f32 = mybir.dt.float32
f32r = mybir.dt.float32r
bf16 = mybir.dt.bfloat16
i32 = mybir.dt.int32
AF = mybir.ActivationFunctionType
OP = mybir.AluOpType
PAIRS = [[0, 1], [2, 3], [4, 5], [6, 7]]

_cache = {}


def _build(nl, stop_after=99, skip=(), tune=()):
    nc = bacc.Bacc(None, num_devices=NCC)

    # ---- DRAM I/O ----
    tok_emb = nc.dram_tensor("tok_emb", [V, D], f32, kind="ExternalInput")
    src_idx = nc.dram_tensor("src_idx", [P, DT], i32, kind="ExternalInput")
    pos = nc.dram_tensor("pos", [T, D], f32, kind="ExternalInput")
    mbt = nc.dram_tensor("mbt", [P, KT], f32, kind="ExternalInput")
    Wq = nc.dram_tensor("Wq", [nl, D, D], bf16, kind="ExternalInput")
    Wk = nc.dram_tensor("Wk", [nl, D, D], bf16, kind="ExternalInput")
    Wv = nc.dram_tensor("Wv", [nl, D, D], bf16, kind="ExternalInput")
    Wo = nc.dram_tensor("Wo", [nl, D, D], bf16, kind="ExternalInput")
    W1 = nc.dram_tensor("W1", [nl, D, F], bf16, kind="ExternalInput")
    W2 = nc.dram_tensor("W2", [nl, F, D], bf16, kind="ExternalInput")
    # per-feature scalars, laid out [128, nl*ntiles] (col = l*ntiles + tile)
    bq_a = nc.dram_tensor("bq_a", [P, nl * DT], f32, kind="ExternalInput")
    bk_a = nc.dram_tensor("bk_a", [P, nl * DT], f32, kind="ExternalInput")
    bo_a = nc.dram_tensor("bo_a", [P, nl * DT], f32, kind="ExternalInput")
    b2_a = nc.dram_tensor("b2_a", [P, nl * DT], f32, kind="ExternalInput")
    g1_a = nc.dram_tensor("g1_a", [P, nl * DT], f32, kind="ExternalInput")
    c1_a = nc.dram_tensor("c1_a", [P, nl * DT], f32, kind="ExternalInput")
    g2_a = nc.dram_tensor("g2_a", [P, nl * DT], f32, kind="ExternalInput")
    c2_a = nc.dram_tensor("c2_a", [P, nl * DT], f32, kind="ExternalInput")
    b1_a = nc.dram_tensor("b1_a", [P, nl * FT], f32, kind="ExternalInput")

    xT_out = nc.dram_tensor("xT_out", [D, T], f32, kind="ExternalOutput")

    with tile.TileContext(nc) as tc, ExitStack() as ctx:
        sb = ctx.enter_context(tc.tile_pool(name="sb", bufs=1))
        pp = ctx.enter_context(tc.tile_pool(name="pp", bufs=1, space="PSUM"))
        dr = ctx.enter_context(tc.tile_pool(name="dr", bufs=1, space="DRAM"))
        sing = ctx.enter_context(tc.tile_pool(name="sing", bufs=1))

        def big(dt=f32r):
            return sb.tile([P, T], dt, name="big", tag="big", bufs=18)

        def c512(shape=(P, 512), dt=bf16, tag="c512", bufs=4):
            return sb.tile(list(shape), dt, name=tag, tag=tag, bufs=bufs)

        def psum(tag="proj", bufs=3, shape=(P, 512)):
            return pp.tile(list(shape), f32, space="PSUM", name=tag, tag=tag, bufs=bufs)

        # ---- constants ----
        ident = sing.tile([P, P], f32)
        make_identity(nc, ident)
        ones_s = sing.tile([P, 1], f32r)
        nc.vector.memset(ones_s.bitcast(f32), 1.0 / D)
        eps_s = sing.tile([1, 1], f32)
        nc.vector.memset(eps_s, EPS)
        mb_s = sing.tile([P, KT], f32)
        nc.sync.dma_start(out=mb_s, in_=mbt[:, :])
        bias = {}
        for name, t_ in (("bq", bq_a), ("bk", bk_a), ("bo", bo_a), ("b2", b2_a),
                         ("g1", g1_a), ("c1", c1_a), ("g2", g2_a), ("c2", c2_a)):
            s = sing.tile([P, nl * DT], f32, name=f"bias_{name}")
            nc.sync.dma_start(out=s, in_=t_[:, :])
            bias[name] = s
        b1_s = sing.tile([P, nl * FT], f32)
        nc.sync.dma_start(out=b1_s, in_=b1_a[:, :])

        # ---- embedding: gather + scale + pos, then transpose to xT ----
        idx_s = sing.tile([P, DT], i32)
        nc.sync.dma_start(out=idx_s, in_=src_idx[:, :])
        x0_tiles = []
        for c in range(DT):
            emb_s = big(f32)
            nc.gpsimd.indirect_dma_start(
                out=emb_s[:, :], out_offset=None,
                in_=tok_emb[:, :],
                in_offset=bass.IndirectOffsetOnAxis(ap=idx_s[:, c:c + 1], axis=0),
            )
            pos_s = big(f32)
            nc.sync.dma_start(out=pos_s, in_=pos[c * P:(c + 1) * P, :])
            x0 = big(f32)
            nc.vector.scalar_tensor_tensor(
                out=x0, in0=emb_s, scalar=float(np.sqrt(D)), in1=pos_s,
                op0=OP.mult, op1=OP.add)
            x0_tiles.append(x0)
        xT = []
        for m in range(DT):
            t_ = big()
            for c in range(DT):
                ps = psum()
                nc.tensor.transpose(ps[:, 0:P], x0_tiles[c][:, m * P:(m + 1) * P], ident)
                nc.vector.tensor_copy(t_[:, c * P:(c + 1) * P], ps[:, 0:P])
            xT.append(t_)

        for l in range(nl if stop_after >= 1 else 0):
            bcol = l * DT

            def wload(wt, mi, n_kt=DT, name="wblk"):
                # weight block [128, n_kt, 128] = wt[l][:, mi*128:(mi+1)*128]
                blk = sb.tile([P, n_kt, P], bf16, name=name, tag="wblk", bufs=3)
                src = wt[l].rearrange("(kt p) c -> p kt c", p=P)[:, :, mi * P:(mi + 1) * P]
                nc.sync.dma_start(out=blk, in_=src)
                return blk

            kvK = dr.tile([D, T], bf16, name="kvK", tag="kvK", bufs=2)
            agK = dr.tile([2 * D, T], bf16, name="agK", tag="agK", bufs=2)
            kvV = dr.tile([T, D], bf16, name="kvV", tag="kvV", bufs=2)
            agV = dr.tile([2 * T, D], bf16, name="agV", tag="agV", bufs=2)

            # ---- K^T projection -> kvK (feature-major, bf16) ----
            for m in range(DT if "kv" not in skip else 0):
                kb = wload(Wk, m)
                ps2 = [psum() for _ in range(NQ)]
                for k in range(DT):
                    for n in range(NQ):
                        nc.tensor.matmul(ps2[n], kb[:, k, :], xT[k][:, n * 512:(n + 1) * 512],
                                         start=(k == 0), stop=(k == DT - 1))
                for n in range(NQ):
                    kchunk = c512()
                    nc.vector.tensor_scalar_add(kchunk, ps2[n], bias["bk"][:, bcol + m:bcol + m + 1])
                    nc.sync.dma_start(out=kvK[m * P:(m + 1) * P, n * 512:(n + 1) * 512],
                                      in_=kchunk)
            # ---- AllGather K^T with same-batch partner (overlaps V/Q proj) ----
            if "kv" not in skip:
                nc.gpsimd.collective_compute(
                    "AllGather", OP.bypass,
                    replica_groups=PAIRS,
                    ins=[kvK[:, :]],
                    outs=[agK[:, :]],
                )
            # ---- V projection -> kvV (token-major, bf16) ----
            vbs = {}
            for nf in range(NQ if "kv" not in skip else 0):
                for k in range(DT):
                    vb = c512(tag="ht", bufs=34)
                    nc.sync.dma_start(out=vb, in_=Wv[l][k * P:(k + 1) * P,
                                                        nf * 512:(nf + 1) * 512])
                    vbs[nf, k] = vb
            for tt in range(DT if "kv" not in skip else 0):
                ps2 = [psum() for _ in range(NQ)]
                for k in range(DT):
                    for nf in range(NQ):
                        nc.tensor.matmul(ps2[nf], xT[k][:, tt * P:(tt + 1) * P], vbs[nf, k],
                                         start=(k == 0), stop=(k == DT - 1))
                for nf in range(NQ):
                    vchunk = c512()
                    nc.vector.tensor_copy(vchunk, ps2[nf])
                    nc.sync.dma_start(out=kvV[tt * P:(tt + 1) * P,
                                              nf * 512:(nf + 1) * 512],
                                      in_=vchunk)
            if stop_after <= 2:
                break
            # ---- AllGather V ----
            if "kv" not in skip:
                nc.gpsimd.collective_compute(
                    "AllGather", OP.bypass,
                    replica_groups=PAIRS,
                    ins=[kvV[:, :]],
                    outs=[agV[:, :]],
                )

            if stop_after <= 3:
                break
            # ---- Q projections for all head pairs (overlap the gathers) ----
            qcs_all = {}
            for hp in range(DT if "attn" not in skip else 0):
                qb = wload(Wq, hp)
                qps = [psum() for _ in range(NQ)]
                for k in range(DT):
                    for qt in range(NQ):
                        nc.tensor.matmul(qps[qt], qb[:, k, :],
                                         xT[k][:, qt * 512:(qt + 1) * 512],
                                         start=(k == 0), stop=(k == DT - 1))
                for qt in range(NQ):
                    qc = c512(tag="qc", bufs=17)
                    nc.vector.tensor_scalar_add(qc, qps[qt], bias["bq"][:, bcol + hp:bcol + hp + 1])
                    qcs_all[hp, qt] = qc

            # ---- attention ----
            attnT = [big() for _ in range(DT)]
            if "attn" in skip:
                for t_ in attnT:
                    nc.vector.memset(t_.bitcast(f32), 0.01)
            for hp in range(DT if "attn" not in skip else 0):  # head pairs
                ktp = sb.tile([P, 2 * T], bf16, name="ktp", tag="ktp", bufs=2)
                nc.sync.dma_start(out=ktp[:, 0:T], in_=agK[ds(hp * P, P), :])
                nc.sync.dma_start(out=ktp[:, T:2 * T], in_=agK[ds(D + hp * P, P), :])
                vags = []
                for hh in range(2):
                    h = 2 * hp + hh
                    va = sb.tile([P, KT, HD + 1], bf16, name="vag", tag="vag", bufs=3)
                    for ri in range(2):
                        src = agV[ds(ri * T, T),
                                  h * HD:(h + 1) * HD].rearrange("(kt p) c -> p kt c", p=P)
                        nc.sync.dma_start(out=va[:, ri * DT:(ri + 1) * DT, 0:HD], in_=src)
                    nc.vector.memset(va[:, :, HD:HD + 1], 1.0)
                    vags.append(va)
                for qt in range(NQ):
                    qsl = slice(qt * 512, (qt + 1) * 512)
                    qc = qcs_all[hp, qt]

                    av_ps = [psum(tag="av", bufs=2, shape=(HD + 1, 512)) for _ in range(2)]
                    prev = None
                    for kt in range(KT):
                        sts = [psum(tag="st", bufs=3) for _ in range(2)]
                        ksl = slice(kt * P, (kt + 1) * P)
                        nc.tensor.matmul(sts[0], ktp[0:HD, ksl], qc[0:HD, :],
                                         start=True, stop=True, tile_position=(0, 0))
                        nc.tensor.matmul(sts[1], ktp[HD:P, ksl], qc[HD:P, :],
                                         start=True, stop=True, tile_position=(64, 0))
                        cur = []
                        for hh in range(2):
                            pr = c512(tag="probs", bufs=4)
                            nc.scalar.activation(out=pr, in_=sts[hh], func=AF.Exp,
                                                 bias=mb_s[:, kt:kt + 1],
                                                 scale=float(1.0 / np.sqrt(HD)))
                            cur.append(pr)
                        # AV lags ST by one kt to hide exp latency
                        if prev is not None:
                            pkt = kt - 1
                            for hh in range(2):
                                nc.tensor.matmul(av_ps[hh], vags[hh][:, pkt, :], prev[hh],
                                                 start=(pkt == 0), stop=False)
                        prev = cur
                    for hh in range(2):
                        nc.tensor.matmul(av_ps[hh], vags[hh][:, KT - 1, :], prev[hh],
                                         start=False, stop=True)
                    for hh in range(2):
                        rs = sb.tile([1, 512], f32, name="rs", tag="rows", bufs=4)
                        nc.vector.reciprocal(rs, av_ps[hh][HD:HD + 1, :])
                        rb = sb.tile([HD, 512], f32, name="rb", tag="rb", bufs=2)
                        nc.gpsimd.partition_broadcast(rb, rs[0:1, :], channels=HD)
                        nc.vector.tensor_tensor(
                            out=attnT[hp][hh * HD:(hh + 1) * HD, qsl],
                            in0=av_ps[hh][0:HD, :], in1=rb, op=OP.mult)

            if stop_after <= 4:
                break
            # ---- O projection + residual (in-place into xT) ; LN1 ----
            for m in range(DT):
                ob = wload(Wo, m)
                ps2 = [psum() for _ in range(NQ)]
                for k in range(DT):
                    for n in range(NQ):
                        nc.tensor.matmul(ps2[n], ob[:, k, :], attnT[k][:, n * 512:(n + 1) * 512],
                                         start=(k == 0), stop=(k == DT - 1))
                for n in range(NQ):
                    nc.vector.scalar_tensor_tensor(
                        out=xT[m][:, n * 512:(n + 1) * 512], in0=ps2[n],
                        scalar=bias["bo"][:, bcol + m:bcol + m + 1],
                        in1=xT[m].bitcast(f32)[:, n * 512:(n + 1) * 512],
                        op0=OP.add, op1=OP.add)
            _layernorm(nc, big, c512, psum, sb, xT, ones_s, eps_s,
                       bias["g1"], bias["c1"], bcol)

            if stop_after <= 5:
                break
            # ---- FFN (F in halves; W1/W2 weights reused across both token chunks) ----
            u2 = []
            if "ffn" in skip:
                for m in range(DT):
                    u2.append(big())
                    nc.vector.tensor_copy(u2[m], xT[m].bitcast(f32))
            for fhalf in range(2 if "ffn" not in skip else 0):
                hT = {}
                for fi in range(FT // 2):
                    fm = fhalf * (FT // 2) + fi
                    w1b = wload(W1, fm)
                    ps2 = [psum() for _ in range(NQ)]
                    for k in range(DT):
                        for tc2 in range(NQ):
                            nc.tensor.matmul(ps2[tc2], w1b[:, k, :],
                                             xT[k][:, tc2 * 512:(tc2 + 1) * 512],
                                             start=(k == 0), stop=(k == DT - 1))
                    for tc2 in range(NQ):
                        ht = c512(tag="ht", bufs=34)
                        nc.vector.tensor_scalar(
                            out=ht, in0=ps2[tc2],
                            scalar1=b1_s[:, l * FT + fm:l * FT + fm + 1], scalar2=0.0,
                            op0=OP.add, op1=OP.max)
                        hT[tc2, fi] = ht
                for m in range(DT):
                    if fhalf == 0:
                        u2.append(big())
                    ps2 = [psum() for _ in range(NQ)]
                    for quarter in range(2):
                        w2b = sb.tile([P, FT // 4, P], bf16, name="wblk", tag="wblk", bufs=3)
                        nc.sync.dma_start(
                            out=w2b,
                            in_=W2[l].rearrange("(kt p) c -> p kt c", p=P)[
                                :, fhalf * (FT // 2) + quarter * (FT // 4):
                                   fhalf * (FT // 2) + (quarter + 1) * (FT // 4),
                                m * P:(m + 1) * P])
                        for kk in range(FT // 4):
                            kl = quarter * (FT // 4) + kk
                            for tc2 in range(NQ):
                                nc.tensor.matmul(ps2[tc2], w2b[:, kk, :], hT[tc2, kl],
                                                 start=(kl == 0), stop=(kl == FT // 2 - 1))
                    for tc2 in range(NQ):
                        tsl = slice(tc2 * 512, (tc2 + 1) * 512)
                        if fhalf == 0:
                            nc.vector.scalar_tensor_tensor(
                                out=u2[m][:, tsl], in0=ps2[tc2],
                                scalar=bias["b2"][:, bcol + m:bcol + m + 1],
                                in1=xT[m].bitcast(f32)[:, tsl],
                                op0=OP.add, op1=OP.add)
                        else:
                            nc.vector.tensor_add(u2[m][:, tsl], ps2[tc2],
                                                 u2[m].bitcast(f32)[:, tsl])
            _layernorm(nc, big, c512, psum, sb, u2, ones_s, eps_s,
                       bias["g2"], bias["c2"], bcol)
            xT = u2

        for m in range(DT):
            nc.sync.dma_start(out=xT_out[m * P:(m + 1) * P, :], in_=xT[m].bitcast(f32))

    nc.finalize()
    return nc


def _layernorm(nc, big, c512, psum, sb, u, ones_s, eps_s, g_s, c_s, bcol):
    """Feature-axis layernorm over feature-major tiles u (list of DT [128, T])."""
    mean_b = big(f32)
    rstd_b = big(f32)
    for n in range(NQ):
        tsl = slice(n * 512, (n + 1) * 512)
        mps = psum()
        for k in range(DT):
            nc.tensor.matmul(mps[0:1, :], ones_s, u[k][:, tsl],
                             start=(k == 0), stop=(k == DT - 1))
        mean_r = sb.tile([1, 512], f32, name="mean_r", tag="rows", bufs=4)
        nc.vector.tensor_copy(mean_r, mps[0:1, :])
        e2ps = psum()
        for k in range(DT):
            usq = c512(tag="probs", bufs=4, dt=f32r)
            nc.vector.tensor_mul(usq, u[k].bitcast(f32)[:, tsl], u[k].bitcast(f32)[:, tsl])
            nc.tensor.matmul(e2ps[0:1, :], ones_s, usq,
                             start=(k == 0), stop=(k == DT - 1))
        var_r = sb.tile([1, 512], f32, name="var_r", tag="rows", bufs=4)
        # var = E[u^2] - mean^2
        nc.vector.tensor_mul(var_r, mean_r, mean_r)
        nc.vector.tensor_sub(var_r, e2ps[0:1, :], var_r)
        sd_r = sb.tile([1, 512], f32, name="sd_r", tag="rows", bufs=4)
        nc.scalar.activation(out=sd_r, in_=var_r, func=AF.Sqrt, bias=eps_s[0:1, :])
        rstd_r = sb.tile([1, 512], f32, name="rstd_r", tag="rows", bufs=4)
        nc.vector.reciprocal(rstd_r, sd_r)
        nc.gpsimd.partition_broadcast(mean_b[:, tsl], mean_r[0:1, :], channels=P)
        nc.gpsimd.partition_broadcast(rstd_b[:, tsl], rstd_r[0:1, :], channels=P)
    for m in range(DT):
        nc.vector.tensor_sub(u[m], u[m].bitcast(f32), mean_b)
        nc.vector.tensor_mul(u[m], u[m].bitcast(f32), rstd_b)
        nc.vector.tensor_scalar(
            out=u[m], in0=u[m].bitcast(f32),
            scalar1=g_s[:, bcol + m:bcol + m + 1],
            scalar2=c_s[:, bcol + m:bcol + m + 1],
            op0=OP.mult, op1=OP.add)
    return u


def _prep_feat(x, nl, nt):
    # [nl, dim] -> [128, nl*nt] with col l*nt + tile
    return np.ascontiguousarray(
        np.asarray(x, np.float32).reshape(nl, nt, P).transpose(2, 0, 1).reshape(P, nl * nt))


def prepare_in_maps(src, src_mask, tok_emb, pos_emb, Wq, bq, Wk, bk, Wv, bv,
                    Wo, bo, ln1_g, ln1_b, W1, b1, W2, b2, ln2_g, ln2_b):
    import ml_dtypes
    nl = np.asarray(Wq).shape[0]
    src = np.asarray(src)
    src_mask = np.asarray(src_mask)
    assert not np.any(np.asarray(bv)), "nonzero bv not implemented"

    np32 = lambda x: np.ascontiguousarray(np.asarray(x, np.float32))
    npbf = lambda x: np.ascontiguousarray(
        np.asarray(x, np.float32).astype(ml_dtypes.bfloat16))
    common = dict(
        tok_emb=np32(tok_emb),
        Wq=npbf(Wq), Wk=npbf(Wk), Wv=npbf(Wv), Wo=npbf(Wo),
        W1=npbf(W1), W2=npbf(W2),
        bq_a=_prep_feat(bq, nl, DT), bk_a=_prep_feat(bk, nl, DT),
        bo_a=_prep_feat(bo, nl, DT), b2_a=_prep_feat(b2, nl, DT),
        g1_a=_prep_feat(ln1_g, nl, DT), c1_a=_prep_feat(ln1_b, nl, DT),
        g2_a=_prep_feat(ln2_g, nl, DT), c2_a=_prep_feat(ln2_b, nl, DT),
        b1_a=_prep_feat(b1, nl, FT),
    )
    in_maps = []
    for c in range(NCC):
        b, hf = c // 2, c % 2
        tok = src[b, hf * T:(hf + 1) * T].astype(np.int32)
        mask = np.asarray(src_mask[b, 0, 0], np.float32)
        mbias = np.where(mask == 0, np.float32(-1e10), np.float32(0.0))
        in_maps.append(dict(
            common,
            src_idx=np.ascontiguousarray(tok.reshape(DT, P).T),
            pos=np32(pos_emb[hf * T:(hf + 1) * T]),
            mbt=np.ascontiguousarray(mbias.reshape(KT, P).T),
        ))
    return nl, in_maps


def assemble_out(per_core_results):
    out = np.empty((B, S, D), np.float32)
    for c in range(NCC):
        b, hf = c // 2, c % 2
        out[b, hf * T:(hf + 1) * T] = per_core_results[c]["xT_out"].T
    return out


def kernel(**inputs):
    nl, in_maps = prepare_in_maps(**inputs)
    if "nc" not in _cache or _cache.get("nl") != nl:
        _cache["nc"] = _build(nl)
        _cache["nl"] = nl
    res = run_bass_kernel_spmd(_cache["nc"], in_maps, core_ids=list(range(NCC)))
    return assemble_out(res.results)


# revision 4
# speedup vs baseline: 55.6815x; 1.4219x over previous
"""Trainium2 Bass kernel for a 6-layer post-LN transformer encoder.

Model: V=32000, D=1024, L=6, H=16, HD=64, F=4096, B=4, S=2048 (fp32).

Sharding: 8 NeuronCores, token-parallel. Core c owns batch b=c//2,
sequence half c%2 (1024 tokens). Per layer, K^T (feature-major) and V
(token-major) for the local tokens are computed locally in bf16, then
exchanged with the same-batch partner core via 2-rank AllGathers
(replica groups [0,1],[2,3],[4,5],[6,7]) — K first so its gather
overlaps the V/Q projections, V's overlaps the score matmuls.

Layouts: activations are feature-major ("x^T": [D partitions, tokens]) so
every projection contracts over partitions. Weights are stored/loaded as
bf16 (halves HBM traffic; PE runs bf16 stationary x f32r moving at full
rate). Scores are computed transposed (S^T = K^T_slice.T @ Q^T_slice per
head, [keys, q]) with both heads of a pair running concurrently via PE
row tiling, softmax sums come from a ones-column appended to V
(attnT_aug = [V|1].T @ P^T), and per-head normalization uses gpsimd
partition_broadcast of the reciprocal sums.
"""
import sys

sys.path.insert(0, "/opt/trn_rl_repo")

from contextlib import ExitStack

import numpy as np

import concourse.bass as bass
import concourse.tile as tile
from concourse import bacc, mybir
from concourse.bass import ds
from concourse.bass_utils import run_bass_kernel_spmd
from concourse.masks import make_identity

P = 128
V, D, L, H, F, M = 32000, 1024, 6, 16, 4096, 2048
B, S = 4, 2048
HD = D // H
EPS = 1e-5
NCC = 8
T = 1024            # tokens per core
DT = D // P         # 8 feature tiles
FT = F // P         # 32 ff tiles
KT = 2048 // P      # 16 key tiles (full batch sequence)
NQ = 2              # 512-token chunks per core
f32 = mybir.dt.float32
f32r = mybir.dt.float32r
bf16 = mybir.dt.bfloat16
i32 = mybir.dt.int32
AF = mybir.ActivationFunctionType
OP = mybir.AluOpType
PAIRS = [[0, 1], [2, 3], [4, 5], [6, 7]]

_cache = {}


def _build(nl, stop_after=99, skip=(), tune=()):
    nc = bacc.Bacc(None, num_devices=NCC)

    # ---- DRAM I/O ----
    tok_emb = nc.dram_tensor("tok_emb", [V, D], f32, kind="ExternalInput")
    src_idx = nc.dram_tensor("src_idx", [P, DT], i32, kind="ExternalInput")
    pos = nc.dram_tensor("pos", [T, D], f32, kind="ExternalInput")
    mbt = nc.dram_tensor("mbt", [P, KT], f32, kind="ExternalInput")
    Wq = nc.dram_tensor("Wq", [nl, D, D], bf16, kind="ExternalInput")
    Wk = nc.dram_tensor("Wk", [nl, D, D], bf16, kind="ExternalInput")
    Wv = nc.dram_tensor("Wv", [nl, D, D], bf16, kind="ExternalInput")
    Wo = nc.dram_tensor("Wo", [nl, D, D], bf16, kind="ExternalInput")
    W1 = nc.dram_tensor("W1", [nl, D, F], bf16, kind="ExternalInput")
    W2 = nc.dram_tensor("W2", [nl, F, D], bf16, kind="ExternalInput")
    # per-feature scalars, laid out [128, nl*ntiles] (col = l*ntiles + tile)
    bq_a = nc.dram_tensor("bq_a", [P, nl * DT], f32, kind="ExternalInput")
    bk_a = nc.dram_tensor("bk_a", [P, nl * DT], f32, kind="ExternalInput")
    bo_a = nc.dram_tensor("bo_a", [P, nl * DT], f32, kind="ExternalInput")
    b2_a = nc.dram_tensor("b2_a", [P, nl * DT], f32, kind="ExternalInput")
    g1_a = nc.dram_tensor("g1_a", [P, nl * DT], f32, kind="ExternalInput")
    c1_a = nc.dram_tensor("c1_a", [P, nl * DT], f32, kind="ExternalInput")
    g2_a = nc.dram_tensor("g2_a", [P, nl * DT], f32, kind="ExternalInput")
    c2_a = nc.dram_tensor("c2_a", [P, nl * DT], f32, kind="ExternalInput")
    b1_a = nc.dram_tensor("b1_a", [P, nl * FT], f32, kind="ExternalInput")

    xT_out = nc.dram_tensor("xT_out", [D, T], f32, kind="ExternalOutput")

    with tile.TileContext(nc) as tc, ExitStack() as ctx:
        sb = ctx.enter_context(tc.tile_pool(name="sb", bufs=1))
        pp = ctx.enter_context(tc.tile_pool(name="pp", bufs=1, space="PSUM"))
        dr = ctx.enter_context(tc.tile_pool(name="dr", bufs=1, space="DRAM"))
        sing = ctx.enter_context(tc.tile_pool(name="sing", bufs=1))

        def big(dt=f32r):
            return sb.tile([P, T], dt, name="big", tag="big", bufs=18)

        def c512(shape=(P, 512), dt=bf16, tag="c512", bufs=4):
            return sb.tile(list(shape), dt, name=tag, tag=tag, bufs=bufs)

        def psum(tag="proj", bufs=3, shape=(P, 512)):
            return pp.tile(list(shape), f32, space="PSUM", name=tag, tag=tag, bufs=bufs)

        # ---- constants ----
        ident = sing.tile([P, P], f32)
        make_identity(nc, ident)
        ones_s = sing.tile([P, 1], f32r)
        nc.vector.memset(ones_s.bitcast(f32), 1.0 / D)
        eps_s = sing.tile([1, 1], f32)
        nc.vector.memset(eps_s, EPS)
        mb_s = sing.tile([P, KT], f32)
        nc.sync.dma_start(out=mb_s, in_=mbt[:, :])
        bias = {}
        for name, t_ in (("bq", bq_a), ("bk", bk_a), ("bo", bo_a), ("b2", b2_a),
                         ("g1", g1_a), ("c1", c1_a), ("g2", g2_a), ("c2", c2_a)):
            s = sing.tile([P, nl * DT], f32, name=f"bias_{name}")
            nc.sync.dma_start(out=s, in_=t_[:, :])
            bias[name] = s
        b1_s = sing.tile([P, nl * FT], f32)
        nc.sync.dma_start(out=b1_s, in_=b1_a[:, :])

        # ---- embedding: gather + scale + pos, then transpose to xT ----
        idx_s = sing.tile([P, DT], i32)
        nc.sync.dma_start(out=idx_s, in_=src_idx[:, :])
        x0_tiles = []
        for c in range(DT):
            emb_s = big(f32)
            nc.gpsimd.indirect_dma_start(
                out=emb_s[:, :], out_offset=None,
                in_=tok_emb[:, :],
                in_offset=bass.IndirectOffsetOnAxis(ap=idx_s[:, c:c + 1], axis=0),
            )
            pos_s = big(f32)
            nc.sync.dma_start(out=pos_s, in_=pos[c * P:(c + 1) * P, :])
            x0 = big(f32)
            nc.vector.scalar_tensor_tensor(
                out=x0, in0=emb_s, scalar=float(np.sqrt(D)), in1=pos_s,
                op0=OP.mult, op1=OP.add)
            x0_tiles.append(x0)
        xT = []
        for m in range(DT):
            t_ = big()
            for c in range(DT):
                ps = psum()
                nc.tensor.transpose(ps[:, 0:P], x0_tiles[c][:, m * P:(m + 1) * P], ident)
                nc.vector.tensor_copy(t_[:, c * P:(c + 1) * P], ps[:, 0:P])
            xT.append(t_)

        def bf_cast(tiles):
            # bf16 shadow copies of feature-major activation tiles for
            # matmul operands (PE rejects mixed f32r x bf16 operands)
            out = []
            for t_ in tiles:
                b_ = sb.tile([P, T], bf16, name="xtb", tag="xtb", bufs=10)
                nc.vector.tensor_copy(b_, t_.bitcast(f32))
                out.append(b_)
            return out

        for l in range(nl if stop_after >= 1 else 0):
            bcol = l * DT

            def wload(wt, mi, n_kt=DT, name="wblk"):
                # weight block [128, n_kt, 128] = wt[l][:, mi*128:(mi+1)*128]
                blk = sb.tile([P, n_kt, P], bf16, name=name, tag="wblk", bufs=3)
                src = wt[l].rearrange("(kt p) c -> p kt c", p=P)[:, :, mi * P:(mi + 1) * P]
                nc.sync.dma_start(out=blk, in_=src)
                return blk

            xb = bf_cast(xT)
            kvK = dr.tile([D, T], bf16, name="kvK", tag="kvK", bufs=2)
            agK = dr.tile([2 * D, T], bf16, name="agK", tag="agK", bufs=2)
            kvV = dr.tile([T, D], bf16, name="kvV", tag="kvV", bufs=2)
            agV = dr.tile([2 * T, D], bf16, name="agV", tag="agV", bufs=2)

            # ---- K^T projection -> kvK (feature-major, bf16) ----
            for m in range(DT if "kv" not in skip else 0):
                kb = wload(Wk, m)
                ps2 = [psum() for _ in range(NQ)]
                for k in range(DT):
                    for n in range(NQ):
                        nc.tensor.matmul(ps2[n], kb[:, k, :], xb[k][:, n * 512:(n + 1) * 512],
                                         start=(k == 0), stop=(k == DT - 1))
                for n in range(NQ):
                    kchunk = c512()
                    nc.vector.tensor_scalar_add(kchunk, ps2[n], bias["bk"][:, bcol + m:bcol + m + 1])
                    nc.sync.dma_start(out=kvK[m * P:(m + 1) * P, n * 512:(n + 1) * 512],
                                      in_=kchunk)
            # ---- AllGather K^T with same-batch partner (overlaps V/Q proj) ----
            if "kv" not in skip:
                nc.gpsimd.collective_compute(
                    "AllGather", OP.bypass,
                    replica_groups=PAIRS,
                    ins=[kvK[:, :]],
                    outs=[agK[:, :]],
                )
            # ---- V projection -> kvV (token-major, bf16) ----
            vbs = {}
            for nf in range(NQ if "kv" not in skip else 0):
                for k in range(DT):
                    vb = c512(tag="ht", bufs=34)
                    nc.sync.dma_start(out=vb, in_=Wv[l][k * P:(k + 1) * P,
                                                        nf * 512:(nf + 1) * 512])
                    vbs[nf, k] = vb
            for tt in range(DT if "kv" not in skip else 0):
                ps2 = [psum() for _ in range(NQ)]
                for k in range(DT):
                    for nf in range(NQ):
                        nc.tensor.matmul(ps2[nf], xb[k][:, tt * P:(tt + 1) * P], vbs[nf, k],
                                         start=(k == 0), stop=(k == DT - 1))
                for nf in range(NQ):
                    vchunk = c512()
                    nc.vector.tensor_copy(vchunk, ps2[nf])
                    nc.sync.dma_start(out=kvV[tt * P:(tt + 1) * P,
                                              nf * 512:(nf + 1) * 512],
                                      in_=vchunk)
            if stop_after <= 2:
                break
            # ---- AllGather V ----
            if "kv" not in skip:
                nc.gpsimd.collective_compute(
                    "AllGather", OP.bypass,
                    replica_groups=PAIRS,
                    ins=[kvV[:, :]],
                    outs=[agV[:, :]],
                )

            if stop_after <= 3:
                break
            # ---- Q projections for all head pairs (overlap the gathers) ----
            qcs_all = {}
            for hp in range(DT if "attn" not in skip else 0):
                qb = wload(Wq, hp)
                qps = [psum() for _ in range(NQ)]
                for k in range(DT):
                    for qt in range(NQ):
                        nc.tensor.matmul(qps[qt], qb[:, k, :],
                                         xb[k][:, qt * 512:(qt + 1) * 512],
                                         start=(k == 0), stop=(k == DT - 1))
                for qt in range(NQ):
                    qc = c512(tag="qc", bufs=17)
                    nc.vector.tensor_scalar_add(qc, qps[qt], bias["bq"][:, bcol + hp:bcol + hp + 1])
                    qcs_all[hp, qt] = qc

            # ---- attention ----
            attnT = [big(bf16) for _ in range(DT)]
            if "attn" in skip:
                for t_ in attnT:
                    nc.vector.memset(t_, 0.01)
            for hp in range(DT if "attn" not in skip else 0):  # head pairs
                ktp = sb.tile([P, 2 * T], bf16, name="ktp", tag="ktp", bufs=2)
                nc.sync.dma_start(out=ktp[:, 0:T], in_=agK[ds(hp * P, P), :])
                nc.sync.dma_start(out=ktp[:, T:2 * T], in_=agK[ds(D + hp * P, P), :])
                vags = []
                for hh in range(2):
                    h = 2 * hp + hh
                    va = sb.tile([P, KT, HD + 1], bf16, name="vag", tag="vag", bufs=3)
                    for ri in range(2):
                        src = agV[ds(ri * T, T),
                                  h * HD:(h + 1) * HD].rearrange("(kt p) c -> p kt c", p=P)
                        nc.sync.dma_start(out=va[:, ri * DT:(ri + 1) * DT, 0:HD], in_=src)
                    nc.vector.memset(va[:, :, HD:HD + 1], 1.0)
                    vags.append(va)
                for qt in range(NQ):
                    qsl = slice(qt * 512, (qt + 1) * 512)
                    qc = qcs_all[hp, qt]

                    av_ps = [psum(tag="av", bufs=2, shape=(HD + 1, 512)) for _ in range(2)]
                    prev = None
                    for kt in range(KT):
                        sts = [psum(tag="st", bufs=3) for _ in range(2)]
                        ksl = slice(kt * P, (kt + 1) * P)
                        nc.tensor.matmul(sts[0], ktp[0:HD, ksl], qc[0:HD, :],
                                         start=True, stop=True, tile_position=(0, 0))
                        nc.tensor.matmul(sts[1], ktp[HD:P, ksl], qc[HD:P, :],
                                         start=True, stop=True, tile_position=(64, 0))
                        cur = []
                        for hh in range(2):
                            pr = c512(tag="probs", bufs=4)
                            nc.scalar.activation(out=pr, in_=sts[hh], func=AF.Exp,
                                                 bias=mb_s[:, kt:kt + 1],
                                                 scale=float(1.0 / np.sqrt(HD)))
                            cur.append(pr)
                        # AV lags ST by one kt to hide exp latency
                        if prev is not None:
                            pkt = kt - 1
                            for hh in range(2):
                                nc.tensor.matmul(av_ps[hh], vags[hh][:, pkt, :], prev[hh],
                                                 start=(pkt == 0), stop=False)
                        prev = cur
                    for hh in range(2):
                        nc.tensor.matmul(av_ps[hh], vags[hh][:, KT - 1, :], prev[hh],
                                         start=False, stop=True)
                    for hh in range(2):
                        rs = sb.tile([1, 512], f32, name="rs", tag="rows", bufs=4)
                        nc.vector.reciprocal(rs, av_ps[hh][HD:HD + 1, :])
                        rb = sb.tile([HD, 512], f32, name="rb", tag="rb", bufs=2)
                        nc.gpsimd.partition_broadcast(rb, rs[0:1, :], channels=HD)
                        nc.vector.tensor_tensor(
                            out=attnT[hp][hh * HD:(hh + 1) * HD, qsl],
                            in0=av_ps[hh][0:HD, :], in1=rb, op=OP.mult)

            if stop_after <= 4:
                break
            # ---- O projection + residual (in-place into xT) ; LN1 ----
            for m in range(DT):
                ob = wload(Wo, m)
                ps2 = [psum() for _ in range(NQ)]
                for k in range(DT):
                    for n in range(NQ):
                        nc.tensor.matmul(ps2[n], ob[:, k, :], attnT[k][:, n * 512:(n + 1) * 512],
                                         start=(k == 0), stop=(k == DT - 1))
                for n in range(NQ):
                    nc.vector.scalar_tensor_tensor(
                        out=xT[m][:, n * 512:(n + 1) * 512], in0=ps2[n],
                        scalar=bias["bo"][:, bcol + m:bcol + m + 1],
                        in1=xT[m].bitcast(f32)[:, n * 512:(n + 1) * 512],
                        op0=OP.add, op1=OP.add)
            _layernorm(nc, big, c512, psum, sb, xT, ones_s, eps_s,
                       bias["g1"], bias["c1"], bcol)

            if stop_after <= 5:
                break
            xb1 = bf_cast(xT)
            # ---- FFN (F in halves; W1/W2 weights reused across both token chunks) ----
            u2 = []
            if "ffn" in skip:
                for m in range(DT):
                    u2.append(big())
                    nc.vector.tensor_copy(u2[m], xT[m].bitcast(f32))
            for fhalf in range(2 if "ffn" not in skip else 0):
                hT = {}
                for fi in range(FT // 2):
                    fm = fhalf * (FT // 2) + fi
                    w1b = wload(W1, fm)
                    ps2 = [psum() for _ in range(NQ)]
                    for k in range(DT):
                        for tc2 in range(NQ):
                            nc.tensor.matmul(ps2[tc2], w1b[:, k, :],
                                             xb1[k][:, tc2 * 512:(tc2 + 1) * 512],
                                             start=(k == 0), stop=(k == DT - 1))
                    for tc2 in range(NQ):
                        ht = c512(tag="ht", bufs=34)
                        nc.vector.tensor_scalar(
                            out=ht, in0=ps2[tc2],
                            scalar1=b1_s[:, l * FT + fm:l * FT + fm + 1], scalar2=0.0,
                            op0=OP.add, op1=OP.max)
                        hT[tc2, fi] = ht
                for m in range(DT):
                    if fhalf == 0:
                        u2.append(big())
                    ps2 = [psum() for _ in range(NQ)]
                    for quarter in range(2):
                        w2b = sb.tile([P, FT // 4, P], bf16, name="wblk", tag="wblk", bufs=3)
                        nc.sync.dma_start(
                            out=w2b,
                            in_=W2[l].rearrange("(kt p) c -> p kt c", p=P)[
                                :, fhalf * (FT // 2) + quarter * (FT // 4):
                                   fhalf * (FT // 2) + (quarter + 1) * (FT // 4),
                                m * P:(m + 1) * P])
                        for kk in range(FT // 4):
                            kl = quarter * (FT // 4) + kk
                            for tc2 in range(NQ):
                                nc.tensor.matmul(ps2[tc2], w2b[:, kk, :], hT[tc2, kl],
                                                 start=(kl == 0), stop=(kl == FT // 2 - 1))
                    for tc2 in range(NQ):
                        tsl = slice(tc2 * 512, (tc2 + 1) * 512)
                        if fhalf == 0:
                            nc.vector.scalar_tensor_tensor(
                                out=u2[m][:, tsl], in0=ps2[tc2],
                                scalar=bias["b2"][:, bcol + m:bcol + m + 1],
                                in1=xT[m].bitcast(f32)[:, tsl],
                                op0=OP.add, op1=OP.add)
                        else:
                            nc.vector.tensor_add(u2[m][:, tsl], ps2[tc2],
                                                 u2[m].bitcast(f32)[:, tsl])
            _layernorm(nc, big, c512, psum, sb, u2, ones_s, eps_s,
                       bias["g2"], bias["c2"], bcol)
            xT = u2

        for m in range(DT):
            nc.sync.dma_start(out=xT_out[m * P:(m + 1) * P, :], in_=xT[m].bitcast(f32))

    nc.finalize()
    return nc


def _layernorm(nc, big, c512, psum, sb, u, ones_s, eps_s, g_s, c_s, bcol):
    """Feature-axis layernorm over feature-major tiles u (list of DT [128, T])."""
    mean_b = big(f32)
    rstd_b = big(f32)
    for n in range(NQ):
        tsl = slice(n * 512, (n + 1) * 512)
        mps = psum()
        for k in range(DT):
            nc.tensor.matmul(mps[0:1, :], ones_s, u[k][:, tsl],
                             start=(k == 0), stop=(k == DT - 1))
        mean_r = sb.tile([1, 512], f32, name="mean_r", tag="rows", bufs=4)
        nc.vector.tensor_copy(mean_r, mps[0:1, :])
        e2ps = psum()
        for k in range(DT):
            usq = c512(tag="probs", bufs=4, dt=f32r)
            nc.vector.tensor_mul(usq, u[k].bitcast(f32)[:, tsl], u[k].bitcast(f32)[:, tsl])
            nc.tensor.matmul(e2ps[0:1, :], ones_s, usq,
                             start=(k == 0), stop=(k == DT - 1))
        var_r = sb.tile([1, 512], f32, name="var_r", tag="rows", bufs=4)
        # var = E[u^2] - mean^2
        nc.vector.tensor_mul(var_r, mean_r, mean_r)
        nc.vector.tensor_sub(var_r, e2ps[0:1, :], var_r)
        sd_r = sb.tile([1, 512], f32, name="sd_r", tag="rows", bufs=4)
        nc.scalar.activation(out=sd_r, in_=var_r, func=AF.Sqrt, bias=eps_s[0:1, :])
        rstd_r = sb.tile([1, 512], f32, name="rstd_r", tag="rows", bufs=4)
        nc.vector.reciprocal(rstd_r, sd_r)
        nc.gpsimd.partition_broadcast(mean_b[:, tsl], mean_r[0:1, :], channels=P)
        nc.gpsimd.partition_broadcast(rstd_b[:, tsl], rstd_r[0:1, :], channels=P)
    for m in range(DT):
        nc.vector.tensor_sub(u[m], u[m].bitcast(f32), mean_b)
        nc.vector.tensor_mul(u[m], u[m].bitcast(f32), rstd_b)
        nc.vector.tensor_scalar(
            out=u[m], in0=u[m].bitcast(f32),
            scalar1=g_s[:, bcol + m:bcol + m + 1],
            scalar2=c_s[:, bcol + m:bcol + m + 1],
            op0=OP.mult, op1=OP.add)
    return u


def _prep_feat(x, nl, nt):
    # [nl, dim] -> [128, nl*nt] with col l*nt + tile
    return np.ascontiguousarray(
        np.asarray(x, np.float32).reshape(nl, nt, P).transpose(2, 0, 1).reshape(P, nl * nt))


def prepare_in_maps(src, src_mask, tok_emb, pos_emb, Wq, bq, Wk, bk, Wv, bv,
                    Wo, bo, ln1_g, ln1_b, W1, b1, W2, b2, ln2_g, ln2_b):
    import ml_dtypes
    nl = np.asarray(Wq).shape[0]
    src = np.asarray(src)
    src_mask = np.asarray(src_mask)
    assert not np.any(np.asarray(bv)), "nonzero bv not implemented"

    np32 = lambda x: np.ascontiguousarray(np.asarray(x, np.float32))
    npbf = lambda x: np.ascontiguousarray(
        np.asarray(x, np.float32).astype(ml_dtypes.bfloat16))
    common = dict(
        tok_emb=np32(tok_emb),
        Wq=npbf(Wq), Wk=npbf(Wk), Wv=npbf(Wv), Wo=npbf(Wo),
        W1=npbf(W1), W2=npbf(W2),
        bq_a=_prep_feat(bq, nl, DT), bk_a=_prep_feat(bk, nl, DT),
        bo_a=_prep_feat(bo, nl, DT), b2_a=_prep_feat(b2, nl, DT),
        g1_a=_prep_feat(ln1_g, nl, DT), c1_a=_prep_feat(ln1_b, nl, DT),
        g2_a=_prep_feat(ln2_g, nl, DT), c2_a=_prep_feat(ln2_b, nl, DT),
        b1_a=_prep_feat(b1, nl, FT),
    )
    in_maps = []
    for c in range(NCC):
        b, hf = c // 2, c % 2
        tok = src[b, hf * T:(hf + 1) * T].astype(np.int32)
        mask = np.asarray(src_mask[b, 0, 0], np.float32)
        mbias = np.where(mask == 0, np.float32(-1e10), np.float32(0.0))
        in_maps.append(dict(
            common,
            src_idx=np.ascontiguousarray(tok.reshape(DT, P).T),
            pos=np32(pos_emb[hf * T:(hf + 1) * T]),
            mbt=np.ascontiguousarray(mbias.reshape(KT, P).T),
        ))
    return nl, in_maps


def assemble_out(per_core_results):
    out = np.empty((B, S, D), np.float32)
    for c in range(NCC):
        b, hf = c // 2, c % 2
        out[b, hf * T:(hf + 1) * T] = per_core_results[c]["xT_out"].T
    return out


def kernel(**inputs):
    nl, in_maps = prepare_in_maps(**inputs)
    if "nc" not in _cache or _cache.get("nl") != nl:
        _cache["nc"] = _build(nl)
        _cache["nl"] = nl
    res = run_bass_kernel_spmd(_cache["nc"], in_maps, core_ids=list(range(NCC)))
    return assemble_out(res.results)
